# revision 3
# baseline (speedup 1.0000x reference)
"""Single-head causal attention, distributed across 8 TRN2 NeuronCores. V2.

Reference computation (fp32):
    Q = x @ Wq.T; K = x @ Wk.T; V = x @ Wv.T        # x [B=4, T=4096, C=768], W* [H=64, C]
    out = softmax(causal(Q @ K.T / sqrt(C))) @ V     # out [B, T, H]

Sharding (load-balanced causal split): 8 cores = 4 batches x 2 core types.
Each batch's T=4096 rows split into 8 chunks of 512; core type A handles
chunks {0,3,4,7}, type B handles {1,2,5,6}. Slot j (one 512-query chunk)
processes NCTX[j] = {4,12,20,28} full "context" k-tiles (keys in natural
order from a shared ctx region) plus 4 "diagonal" k-tiles (the slot's own
rows, causally masked with static triangular masks). Slots where a core
needs fewer context tiles than the program provides are killed by a
per-core gate bias (-50) folded into the exp. Total per core: 80 k-tiles
of 128x512 vs. 104 for the naive q-half split.

Data path is bf16 (fp32 PSUM accumulation): halves input DMA (8.6 MB/core)
and SBUF traffic. Scores are computed transposed, St[k,q] = Kt_tile.T @ Qt,
with K stored double-deck (even tiles in partitions 0:64, odd in 64:128) so
St tile pairs row-pack the PE array (two concurrent 64-contract matmuls via
tile_position row groups). exp() runs 1024 wide (2 k-tiles / instruction).
P tiles feed P@V directly with V|ones [128, 65] stationary; the ones column
accumulates the softmax denominator in the same PSUM accumulation.
"""

import numpy as np

B, T, C, H = 4, 4096, 768, 64
NSLOT = 4                 # 512-query slots per core
NCTX = (4, 12, 20, 28)    # context k-tiles per slot (program, uniform)
CTXT = 28                 # total ctx k-tiles (rows 0:3584, natural order)
NOWN = 4                  # own 512-row chunks per core
TOWN = NOWN * 512         # own rows per core
TCTX = CTXT * 128         # ctx rows staged
XCOLS = TOWN + TCTX       # staged xT columns per core
NDIAG = 4 * NSLOT         # diag k-tiles (4 per slot)
CPAIRS = tuple(n // 2 for n in NCTX)   # ctx St pairs per slot
GOFF = (0, 2, 8, 18)      # gate column offset per slot (cumsum CPAIRS)
NGATE = 32
CHUNKS = ((0, 3, 4, 7), (1, 2, 5, 6))  # chunk ids per core type
SCALE = float(C) ** -0.5


DEFAULT_CFG = dict(
    structure="pipe",    # "pipe" | "split" | "inter"
    psum_share=False,    # proj/pst2 share the stp psum tag
    stp_bufs=2,
    pexp_tags=16,
    pexp_bufs=1,
    diag_first=False,
)


def build_bass(niter: int = 1, **cfg_over):
    cfg = dict(DEFAULT_CFG)
    cfg.update(cfg_over)
    import concourse.bacc as bacc
    import concourse.mybir as mybir
    from concourse import tile
    from concourse.masks import make_identity

    fp32 = mybir.dt.float32
    bf16 = mybir.dt.bfloat16
    Exp = mybir.ActivationFunctionType.Exp

    nc = bacc.Bacc("TRN2", target_bir_lowering=False, num_devices=8)
    xT_d = nc.dram_tensor("xT", [C, XCOLS], bf16, kind="ExternalInput")
    wT_d = nc.dram_tensor("wT", [C, 4 * H], bf16, kind="ExternalInput")
    gate_d = nc.dram_tensor("gate", [128, NGATE], fp32, kind="ExternalInput")
    out_d = nc.dram_tensor("out", [TOWN, H], fp32, kind="ExternalOutput")

    xT_ap = xT_d.ap().rearrange("(a p) n -> p a n", p=128)

    with tile.TileContext(nc) as tc:
        with (
            tc.tile_pool(name="const", bufs=1) as constp,
            tc.tile_pool(name="data", bufs=1) as datap,
            tc.tile_pool(name="xtp", bufs=1) as xtp,
            tc.tile_pool(name="pep", bufs=1) as pep,
            tc.tile_pool(name="work", bufs=3) as workp,
            tc.tile_pool(name="ps", bufs=2, space="PSUM") as psp,
        ):
            def body(_iv=None):
                # --- constants ---
                w_sb = constp.tile([128, 6, 4 * H], bf16, tag="w")
                nc.sync.dma_start(
                    w_sb[:], wT_d.ap().rearrange("(a p) n -> p a n", p=128)
                )
                gate_sb = constp.tile([128, NGATE], fp32, tag="gate")
                idb = constp.tile([128, 128], bf16, tag="idb")
                make_identity(nc, idb[:])
                idf = constp.tile([H + 1, H + 1], fp32, tag="idf")
                make_identity(nc, idf[:])


                # --- input staging DMAs (issued in consumption order:
                # interleaved by slot needs) ---
                xown = [None] * NOWN
                xctx = [None] * 7
                def dma_own(oc):
                    xt = xtp.tile([128, 6, 512], bf16, tag=f"xo{oc}")
                    nc.sync.dma_start(
                        xt[:], xT_ap[:, :, 512 * oc : 512 * (oc + 1)]
                    )
                    xown[oc] = xt
                def dma_ctx(cc):
                    xt = xtp.tile([128, 6, 512], bf16, tag=f"xc{cc}")
                    nc.sync.dma_start(
                        xt[:],
                        xT_ap[:, :, TOWN + 512 * cc : TOWN + 512 * (cc + 1)],
                    )
                    xctx[cc] = xt
                dma_own(0); dma_ctx(0)
                nc.sync.dma_start(gate_sb[:], gate_d.ap())
                dma_own(1); dma_ctx(1); dma_ctx(2)
                dma_own(2); dma_ctx(3); dma_ctx(4)
                dma_own(3); dma_ctx(5); dma_ctx(6)

                # --- persistent on-chip tensors ---
                # K double-deck: pair column p holds tile 2p in partitions
                # 0:64 and tile 2p+1 in partitions 64:128.
                ktc = datap.tile([128, (CTXT // 2) * 128], bf16, tag="ktc")
                ktd = datap.tile([128, (NDIAG // 2) * 128], bf16, tag="ktd")
                qt = datap.tile([128, TOWN], bf16, tag="qt")
                vones = datap.tile([128, CTXT + NDIAG, H + 1], bf16, tag="vones")
                nc.vector.memset(vones[:, :, H : H + 1], 1.0)

                proj_ctr = [0]

                def proj_tile():
                    if cfg["psum_share"]:
                        return psp.tile([128, 1024], fp32, tag="stp",
                                        bufs=cfg["stp_bufs"], name="prj")
                    proj_ctr[0] ^= 1
                    return psp.tile([128, 512], fp32,
                                    tag=f"proj{proj_ctr[0]}", bufs=1,
                                    name="prj")

                def kv_proj(xt, wcol0, kt, pcol0, vtile0, vtag):
                    """Project one 512-col x chunk into K double-deck + V|ones."""
                    pskv = proj_tile()
                    for ct in range(6):
                        nc.tensor.matmul(
                            pskv[:, 0:512],
                            lhsT=w_sb[:, ct, wcol0 : wcol0 + 128],
                            rhs=xt[:, ct, :],
                            start=(ct == 0), stop=(ct == 5),
                        )
                    # K cols j=2h+deck of pskv -> deck row-half, pair col h
                    ksrc = pskv[0:64, 0:512].rearrange(
                        "p (h two x) -> p two h x", two=2, x=128
                    )
                    for deck in range(2):
                        nc.vector.tensor_copy(
                            kt[64 * deck : 64 * (deck + 1),
                               128 * pcol0 : 128 * (pcol0 + 2)].rearrange(
                                "p (h x) -> p h x", x=128
                            ),
                            ksrc[:, deck, :, :],
                        )
                    vt = workp.tile([64, 512], bf16, tag=vtag)
                    nc.vector.tensor_copy(vt[:], pskv[64:128, 0:512])
                    pst = psp.tile([128, 4, H], bf16, tag="ptr", bufs=1)
                    for j in range(4):
                        nc.tensor.transpose(
                            pst[:, j, :], vt[:, 128 * j : 128 * (j + 1)],
                            idb[0:64, 0:64],
                        )
                    nc.vector.tensor_copy(
                        vones[:, vtile0 : vtile0 + 4, 0:H], pst[:]
                    )

                def q_proj(oc):
                    psq = proj_tile()
                    for ct in range(6):
                        nc.tensor.matmul(
                            psq[:, 0:512],
                            lhsT=w_sb[:, ct, 0 : 2 * H],
                            rhs=xown[oc][:, ct, :],
                            start=(ct == 0), stop=(ct == 5),
                        )
                    nc.vector.tensor_copy(
                        qt[:, 512 * oc : 512 * (oc + 1)], psq[:, 0:512]
                    )

                def own_proj(oc):
                    q_proj(oc)
                    kv_proj(xown[oc], 2 * H, ktd, 2 * oc, CTXT + 4 * oc, "vtd")

                def ctx_proj(cc):
                    kv_proj(xctx[cc], 2 * H, ktc, 2 * cc, 4 * cc, "vtc")

                pv_tiles = {}
                pexp_store = {}

                def st_pair_emitters(j):
                    """One emitter per pair of slot j: St -> exp (-> mask)."""
                    qs = slice(512 * j, 512 * (j + 1))
                    npairs = CPAIRS[j] + 2
                    if cfg["diag_first"]:
                        order = (list(range(CPAIRS[j], npairs))
                                 + list(range(CPAIRS[j])))
                    else:
                        order = list(range(npairs))
                    pexp_store[j] = [None] * npairs

                    def emit(i, p):
                        diag = p >= CPAIRS[j]
                        kt, pcol = (
                            (ktc, p) if not diag
                            else (ktd, 2 * j + (p - CPAIRS[j]))
                        )
                        stp = psp.tile([128, 1024], fp32, tag="stp",
                                       bufs=cfg["stp_bufs"])
                        for deck in range(2):
                            nc.tensor.matmul(
                                stp[:, 512 * deck : 512 * (deck + 1)],
                                lhsT=kt[64 * deck : 64 * (deck + 1),
                                        128 * pcol : 128 * (pcol + 1)],
                                rhs=qt[64 * deck : 64 * (deck + 1), qs],
                                start=True, stop=True, skip_group_check=True,
                            )
                        pexp = pep.tile(
                            [128, 1024], bf16,
                            tag=f"pexp{(j % 2) * 16 + i % cfg['pexp_tags']}",
                            bufs=cfg["pexp_bufs"],
                        )
                        nc.scalar.activation(
                            pexp[:], stp[:], Exp,
                            bias=(0.0 if diag
                                  else gate_sb[:, GOFF[j] + p : GOFF[j] + p + 1]),
                        )
                        if diag:
                            dd = 2 * (p - CPAIRS[j])
                            for deck in range(2):
                                # causal mask: keep iff qj >= 128*d + ki
                                nc.gpsimd.affine_select(
                                    out=pexp[:, 512 * deck : 512 * (deck + 1)],
                                    in_=pexp[:, 512 * deck : 512 * (deck + 1)],
                                    compare_op=mybir.AluOpType.is_ge,
                                    fill=0.0, base=-128 * (dd + deck),
                                    channel_multiplier=-1, pattern=[[1, 512]],
                                )
                        pexp_store[j][i] = (pexp, p, diag)

                    return [
                        (lambda i=i, p=p: emit(i, p)) for i, p in enumerate(order)
                    ]

                def pv_emitters(j):
                    """One emitter per pair of slot j: P @ [V|ones] accum."""
                    npairs = CPAIRS[j] + 2

                    def emit(i):
                        if i == 0:
                            pv_tiles[j] = psp.tile([H + 1, 512], fp32,
                                                   tag="pv", bufs=1, name="pv")
                        pv = pv_tiles[j]
                        pexp, p, diag = pexp_store[j][i]
                        t0 = (2 * p if not diag
                              else CTXT + 4 * j + 2 * (p - CPAIRS[j]))
                        for deck in range(2):
                            nc.tensor.matmul(
                                pv[:],
                                lhsT=vones[:, t0 + deck, :],
                                rhs=pexp[:, 512 * deck : 512 * (deck + 1)],
                                start=(i == 0 and deck == 0),
                                stop=(i == npairs - 1 and deck == 1),
                                skip_group_check=True,
                            )

                    return [(lambda i=i: emit(i)) for i in range(npairs)]

                def finalize(j):
                    # normalize + transpose back to [q, h]
                    pv = pv_tiles[j]
                    pvs = workp.tile([H + 1, 512], fp32, tag="pvs")
                    nc.vector.tensor_copy(pvs[:], pv[:])
                    for g in range(4):
                        if cfg["psum_share"]:
                            pst2 = psp.tile([128, 1024], fp32, tag="stp",
                                            bufs=cfg["stp_bufs"])
                        else:
                            pst2 = psp.tile([128, H + 1], fp32, tag="ptr",
                                            bufs=1)
                        nc.tensor.transpose(
                            pst2[:, 0 : H + 1], pvs[:, 128 * g : 128 * (g + 1)],
                            idf[:],
                        )
                        rec = workp.tile([128, 1], fp32, tag="rec")
                        nc.vector.reciprocal(rec[:], pst2[:, H : H + 1])
                        outt = workp.tile([128, H], fp32, tag="outt")
                        nc.vector.tensor_scalar_mul(outt[:], pst2[:, 0:H], rec[:])
                        r0 = 512 * j + 128 * g
                        nc.sync.dma_start(out_d.ap()[r0 : r0 + 128, :], outt[:])

                def run_seq(*streams):
                    """Round-robin emit from several emitter lists."""
                    streams = [list(s) for s in streams]
                    while any(streams):
                        for s in streams:
                            if s:
                                s.pop(0)()

                if cfg["structure"] == "pipe":
                    # software-pipelined: slot j's PV interleaves with slot
                    # j+1's St/exp stream so ACT never starves; the last
                    # slot's PV interleaves into its own St tail.
                    own_proj(0); ctx_proj(0)
                    run_seq(st_pair_emitters(0))
                    own_proj(1); ctx_proj(1); ctx_proj(2)
                    run_seq(pv_emitters(0), st_pair_emitters(1))
                    finalize(0)
                    own_proj(2); ctx_proj(3); ctx_proj(4)
                    run_seq(pv_emitters(1), st_pair_emitters(2))
                    finalize(1)
                    own_proj(3); ctx_proj(5); ctx_proj(6)
                    st3 = st_pair_emitters(3)
                    n2 = len(pv_emitters(2))
                    run_seq(pv_emitters(2), st3[:n2])
                    finalize(2)
                    run_seq(pv_emitters(3), st3[n2:])
                    finalize(3)
                else:
                    def slot(j):
                        sts = st_pair_emitters(j)
                        pvs_ = pv_emitters(j)
                        if cfg["structure"] == "inter":
                            run_seq(sts, pvs_)
                        else:
                            run_seq(sts)
                            run_seq(pvs_)
                        finalize(j)

                    own_proj(0); ctx_proj(0)
                    slot(0)
                    own_proj(1); ctx_proj(1); ctx_proj(2)
                    slot(1)
                    own_proj(2); ctx_proj(3); ctx_proj(4)
                    slot(2)
                    own_proj(3); ctx_proj(5); ctx_proj(6)
                    slot(3)

            if niter == 1:
                body()
            else:
                with tc.For_i(0, niter) as iv:
                    body(iv)

    nc.compile()
    return nc


_NC_CACHE = {}


def _get_nc(niter: int = 1):
    if niter not in _NC_CACHE:
        _NC_CACHE[niter] = build_bass(niter)
    return _NC_CACHE[niter]


def make_in_maps(x, Wq, Wk, Wv):
    import ml_dtypes

    bf16 = ml_dtypes.bfloat16
    x = np.asarray(x, np.float32)
    wqs = np.asarray(Wq).T.astype(np.float32) * SCALE
    wT = np.concatenate(
        [wqs, wqs, np.asarray(Wk).T, np.asarray(Wv).T], axis=1
    ).astype(bf16)
    wT = np.ascontiguousarray(wT)

    # gate columns (pair-granular, per core type)
    need = [[4 * c for c in CHUNKS[t]] for t in range(2)]
    gates = []
    for t in range(2):
        g = np.zeros((128, NGATE), np.float32)
        for j in range(NSLOT):
            for p in range(CPAIRS[j]):
                if 2 * p >= need[t][j]:
                    g[:, GOFF[j] + p] = -50.0
        gates.append(np.ascontiguousarray(g))

    in_maps = []
    for c in range(8):
        b, t = c // 2, c % 2
        xT = np.empty((C, XCOLS), bf16)
        for j, ch in enumerate(CHUNKS[t]):
            xT[:, 512 * j : 512 * (j + 1)] = x[b, 512 * ch : 512 * (ch + 1), :].T
        xT[:, TOWN:] = x[b, 0:TCTX, :].T
        in_maps.append(
            {"xT": np.ascontiguousarray(xT), "wT": wT, "gate": gates[t]}
        )
    return in_maps


def kernel(x, Wq, Wk, Wv):
    from concourse.bass_utils import run_bass_kernel_spmd

    x = np.asarray(x, np.float32)
    nc = _get_nc(1)
    in_maps = make_in_maps(x, np.asarray(Wq), np.asarray(Wk), np.asarray(Wv))
    res = run_bass_kernel_spmd(nc, in_maps, core_ids=list(range(8)), trace=False)
    out = np.empty((B, T, H), np.float32)
    for c in range(8):
        b, t = c // 2, c % 2
        r = res.results[c]["out"]
        for j, ch in enumerate(CHUNKS[t]):
            out[b, 512 * ch : 512 * (ch + 1), :] = r[512 * j : 512 * (j + 1)]
    return out


# revision 6
# speedup vs baseline: 1.2472x; 1.2472x over previous
"""Single-head causal attention, distributed across 8 TRN2 NeuronCores. V2.

Reference computation (fp32):
    Q = x @ Wq.T; K = x @ Wk.T; V = x @ Wv.T        # x [B=4, T=4096, C=768], W* [H=64, C]
    out = softmax(causal(Q @ K.T / sqrt(C))) @ V     # out [B, T, H]

Sharding (load-balanced causal split): 8 cores = 4 batches x 2 core types.
Each batch's T=4096 rows split into 8 chunks of 512; core type A handles
chunks {0,3,4,7}, type B handles {1,2,5,6}. Slot j (one 512-query chunk)
processes NCTX[j] = {4,12,20,28} full "context" k-tiles (keys in natural
order from a shared ctx region) plus 4 "diagonal" k-tiles (the slot's own
rows, causally masked with static triangular masks). Slots where a core
needs fewer context tiles than the program provides are killed by a
per-core gate bias (-50) folded into the exp. Total per core: 80 k-tiles
of 128x512 vs. 104 for the naive q-half split.

Data path is bf16 (fp32 PSUM accumulation): halves input DMA (8.6 MB/core)
and SBUF traffic. Scores are computed transposed, St[k,q] = Kt_tile.T @ Qt,
with K stored double-deck (even tiles in partitions 0:64, odd in 64:128) so
St tile pairs row-pack the PE array (two concurrent 64-contract matmuls via
tile_position row groups). exp() runs 1024 wide (2 k-tiles / instruction).
P tiles feed P@V directly with V|ones [128, 65] stationary; the ones column
accumulates the softmax denominator in the same PSUM accumulation.

Emission is software-pipelined: slot j's P@V matmuls round-robin with slot
j+1's St/exp stream (projection groups injected inline at their dependency
points), so the scalar engine -- the scarcest resource -- stays fed; the
last slot's P@V interleaves into its own St tail. Input DMAs are issued in
consumption order so attention starts ~6 us into the 8.6 MB input stream.
"""

import numpy as np

B, T, C, H = 4, 4096, 768, 64
NSLOT = 4                 # 512-query slots per core
NCTX = (4, 12, 20, 28)    # context k-tiles per slot (program, uniform)
CTXT = 28                 # total ctx k-tiles (rows 0:3584, natural order)
NOWN = 4                  # own 512-row chunks per core
TOWN = NOWN * 512         # own rows per core
TCTX = CTXT * 128         # ctx rows staged
XCOLS = TOWN + TCTX       # staged xT columns per core
NDIAG = 4 * NSLOT         # diag k-tiles (4 per slot)
CPAIRS = tuple(n // 2 for n in NCTX)   # ctx St pairs per slot
GOFF = (0, 2, 8, 18)      # gate column offset per slot (cumsum CPAIRS)
NGATE = 32
CHUNKS = ((0, 3, 4, 7), (1, 2, 5, 6))  # chunk ids per core type
SCALE = float(C) ** -0.5


DEFAULT_CFG = dict(
    structure="pipe",    # "pipe" | "split" | "inter"
    psum_share=False,    # proj/pst2 share the stp psum tag
    stp_bufs=2,
    pexp_tags=16,
    pexp_bufs=1,
    diag_first=False,
    deck=True,        # K double-deck + row-packed St pairs
    vtrans="pe",      # "pe": TensorE transpose; "dma": x-bar transpose DMA
    merge_proj=True,  # inject proj groups inline into the St streams
)


def build_bass(niter: int = 1, **cfg_over):
    cfg = dict(DEFAULT_CFG)
    cfg.update(cfg_over)
    import concourse.bacc as bacc
    import concourse.mybir as mybir
    from concourse import tile
    from concourse.masks import make_identity

    fp32 = mybir.dt.float32
    bf16 = mybir.dt.bfloat16
    Exp = mybir.ActivationFunctionType.Exp

    nc = bacc.Bacc("TRN2", target_bir_lowering=False, num_devices=8)
    xT_d = nc.dram_tensor("xT", [C, XCOLS], bf16, kind="ExternalInput")
    wT_d = nc.dram_tensor("wT", [C, 4 * H], bf16, kind="ExternalInput")
    gate_d = nc.dram_tensor("gate", [128, NGATE], fp32, kind="ExternalInput")
    out_d = nc.dram_tensor("out", [TOWN, H], fp32, kind="ExternalOutput")

    xT_ap = xT_d.ap().rearrange("(a p) n -> p a n", p=128)

    with tile.TileContext(nc) as tc:
        with (
            tc.tile_pool(name="const", bufs=1) as constp,
            tc.tile_pool(name="data", bufs=1) as datap,
            tc.tile_pool(name="xtp", bufs=1) as xtp,
            tc.tile_pool(name="pep", bufs=1) as pep,
            tc.tile_pool(name="work", bufs=3) as workp,
            tc.tile_pool(name="ps", bufs=2, space="PSUM") as psp,
        ):
            def body(_iv=None):
                # --- constants ---
                w_sb = constp.tile([128, 6, 4 * H], bf16, tag="w")
                nc.sync.dma_start(
                    w_sb[:], wT_d.ap().rearrange("(a p) n -> p a n", p=128)
                )
                gate_sb = constp.tile([128, NGATE], fp32, tag="gate")
                idb = constp.tile([128, 128], bf16, tag="idb")
                make_identity(nc, idb[:])
                idf = constp.tile([H + 1, H + 1], fp32, tag="idf")
                make_identity(nc, idf[:])
                # prewarm the exp table set during the initial DMA wait
                warm = constp.tile([128, 1], fp32, tag="warm")
                nc.scalar.activation(warm[:], warm[:], Exp)


                # --- input staging DMAs (issued in consumption order:
                # interleaved by slot needs) ---
                xown = [None] * NOWN
                xctx = [None] * 7
                def dma_own(oc):
                    xt = xtp.tile([128, 6, 512], bf16, tag=f"xo{oc}")
                    nc.sync.dma_start(
                        xt[:], xT_ap[:, :, 512 * oc : 512 * (oc + 1)]
                    )
                    xown[oc] = xt
                def dma_ctx(cc):
                    xt = xtp.tile([128, 6, 512], bf16, tag=f"xc{cc}")
                    nc.sync.dma_start(
                        xt[:],
                        xT_ap[:, :, TOWN + 512 * cc : TOWN + 512 * (cc + 1)],
                    )
                    xctx[cc] = xt
                dma_own(0); dma_ctx(0)
                nc.sync.dma_start(gate_sb[:], gate_d.ap())
                dma_own(1); dma_ctx(1); dma_ctx(2)
                dma_own(2); dma_ctx(3); dma_ctx(4)
                dma_own(3); dma_ctx(5); dma_ctx(6)

                # --- persistent on-chip tensors ---
                # K double-deck: pair column p holds tile 2p in partitions
                # 0:64 and tile 2p+1 in partitions 64:128.
                if cfg["deck"]:
                    ktc = datap.tile([128, (CTXT // 2) * 128], bf16, tag="ktc")
                    ktd = datap.tile([128, (NDIAG // 2) * 128], bf16, tag="ktd")
                else:
                    ktc = datap.tile([64, CTXT * 128], bf16, tag="ktc")
                    ktd = datap.tile([64, NDIAG * 128], bf16, tag="ktd")
                qt = datap.tile([128, TOWN], bf16, tag="qt")
                # vones pitch: 65 cols used; pad to 80 (160 B) for the 32 B
                # alignment the x-bar transpose DMA needs.
                vpitch = 80 if cfg["vtrans"] == "dma" else H + 1
                vones = datap.tile([128, CTXT + NDIAG, vpitch], bf16,
                                   tag="vones")
                nc.vector.memset(vones[:, :, H : H + 1], 1.0)

                proj_ctr = [0]

                def proj_tile():
                    if cfg["psum_share"]:
                        return psp.tile([128, 1024], fp32, tag="stp",
                                        bufs=cfg["stp_bufs"], name="prj")
                    proj_ctr[0] ^= 1
                    return psp.tile([128, 512], fp32,
                                    tag=f"proj{proj_ctr[0]}", bufs=1,
                                    name="prj")

                def kv_proj(xt, wcol0, kt, pcol0, vtile0, vtag):
                    """Project one 512-col x chunk into K double-deck + V|ones."""
                    pskv = proj_tile()
                    for ct in range(6):
                        nc.tensor.matmul(
                            pskv[:, 0:512],
                            lhsT=w_sb[:, ct, wcol0 : wcol0 + 128],
                            rhs=xt[:, ct, :],
                            start=(ct == 0), stop=(ct == 5),
                        )
                    if cfg["deck"]:
                        # K cols j=2h+deck of pskv -> deck row-half, pair col h
                        ksrc = pskv[0:64, 0:512].rearrange(
                            "p (h two x) -> p two h x", two=2, x=128
                        )
                        for deck in range(2):
                            nc.vector.tensor_copy(
                                kt[64 * deck : 64 * (deck + 1),
                                   128 * pcol0 : 128 * (pcol0 + 2)].rearrange(
                                    "p (h x) -> p h x", x=128
                                ),
                                ksrc[:, deck, :, :],
                            )
                    else:
                        nc.vector.tensor_copy(
                            kt[0:64, 256 * pcol0 : 256 * pcol0 + 512],
                            pskv[0:64, 0:512],
                        )
                    vt = workp.tile([64, 512], bf16, tag=vtag)
                    nc.vector.tensor_copy(vt[:], pskv[64:128, 0:512])
                    if cfg["vtrans"] == "dma":
                        for j in range(4):
                            nc.sync.dma_start_transpose(
                                vones[:, vtile0 + j, 0:H],
                                vt[:, 128 * j : 128 * (j + 1)],
                            )
                    else:
                        pst = psp.tile([128, 4, H], bf16, tag="ptr", bufs=1)
                        for j in range(4):
                            nc.tensor.transpose(
                                pst[:, j, :], vt[:, 128 * j : 128 * (j + 1)],
                                idb[0:64, 0:64],
                            )
                        nc.vector.tensor_copy(
                            vones[:, vtile0 : vtile0 + 4, 0:H], pst[:]
                        )

                def q_proj(oc):
                    psq = proj_tile()
                    for ct in range(6):
                        nc.tensor.matmul(
                            psq[:, 0:512],
                            lhsT=w_sb[:, ct, 0 : 2 * H],
                            rhs=xown[oc][:, ct, :],
                            start=(ct == 0), stop=(ct == 5),
                        )
                    nc.vector.tensor_copy(
                        qt[:, 512 * oc : 512 * (oc + 1)], psq[:, 0:512]
                    )

                def own_proj(oc):
                    q_proj(oc)
                    kv_proj(xown[oc], 2 * H, ktd, 2 * oc, CTXT + 4 * oc, "vtd")

                def ctx_proj(cc):
                    kv_proj(xctx[cc], 2 * H, ktc, 2 * cc, 4 * cc, "vtc")

                pv_tiles = {}
                pexp_store = {}

                def st_pair_emitters(j):
                    """One emitter per pair of slot j: St -> exp (-> mask)."""
                    qs = slice(512 * j, 512 * (j + 1))
                    npairs = CPAIRS[j] + 2
                    if cfg["diag_first"]:
                        order = (list(range(CPAIRS[j], npairs))
                                 + list(range(CPAIRS[j])))
                    else:
                        order = list(range(npairs))
                    pexp_store[j] = [None] * npairs

                    def emit(i, p):
                        diag = p >= CPAIRS[j]
                        kt, pcol = (
                            (ktc, p) if not diag
                            else (ktd, 2 * j + (p - CPAIRS[j]))
                        )
                        stp = psp.tile([128, 1024], fp32, tag="stp",
                                       bufs=cfg["stp_bufs"])
                        for deck in range(2):
                            if cfg["deck"]:
                                lhsT = kt[64 * deck : 64 * (deck + 1),
                                          128 * pcol : 128 * (pcol + 1)]
                                rhs = qt[64 * deck : 64 * (deck + 1), qs]
                            else:
                                t = 2 * pcol + deck
                                lhsT = kt[0:64, 128 * t : 128 * (t + 1)]
                                rhs = qt[0:64, qs]
                            nc.tensor.matmul(
                                stp[:, 512 * deck : 512 * (deck + 1)],
                                lhsT=lhsT, rhs=rhs,
                                start=True, stop=True, skip_group_check=True,
                            )
                        pexp = pep.tile(
                            [128, 1024], bf16,
                            tag=f"pexp{(j % 2) * 16 + i % cfg['pexp_tags']}",
                            bufs=cfg["pexp_bufs"],
                        )
                        nc.scalar.activation(
                            pexp[:], stp[:], Exp,
                            bias=(0.0 if diag
                                  else gate_sb[:, GOFF[j] + p : GOFF[j] + p + 1]),
                        )
                        if diag:
                            dd = 2 * (p - CPAIRS[j])
                            # causal mask over both decks at once:
                            # keep iff qj >= 128*(dd+deck) + ki
                            pex2 = pexp[:].rearrange("p (two q) -> p two q",
                                                     two=2)
                            nc.gpsimd.affine_select(
                                out=pex2, in_=pex2,
                                compare_op=mybir.AluOpType.is_ge,
                                fill=0.0, base=-128 * dd,
                                channel_multiplier=-1,
                                pattern=[[-128, 2], [1, 512]],
                            )
                        pexp_store[j][i] = (pexp, p, diag)

                    return [
                        (lambda i=i, p=p: emit(i, p)) for i, p in enumerate(order)
                    ]

                def pv_emitters(j):
                    """One emitter per pair of slot j: P @ [V|ones] accum."""
                    npairs = CPAIRS[j] + 2

                    def emit(i):
                        if i == 0:
                            pv_tiles[j] = psp.tile([H + 1, 512], fp32,
                                                   tag="pv", bufs=1, name="pv")
                        pv = pv_tiles[j]
                        pexp, p, diag = pexp_store[j][i]
                        t0 = (2 * p if not diag
                              else CTXT + 4 * j + 2 * (p - CPAIRS[j]))
                        for deck in range(2):
                            nc.tensor.matmul(
                                pv[:],
                                lhsT=vones[:, t0 + deck, 0 : H + 1],
                                rhs=pexp[:, 512 * deck : 512 * (deck + 1)],
                                start=(i == 0 and deck == 0),
                                stop=(i == npairs - 1 and deck == 1),
                                skip_group_check=True,
                            )

                    return [(lambda i=i: emit(i)) for i in range(npairs)]

                def finalize(j):
                    # normalize + transpose back to [q, h]
                    pv = pv_tiles[j]
                    pvs = workp.tile([H + 1, 512], fp32, tag="pvs")
                    nc.vector.tensor_copy(pvs[:], pv[:])
                    for g in range(4):
                        if cfg["psum_share"]:
                            pst2 = psp.tile([128, 1024], fp32, tag="stp",
                                            bufs=cfg["stp_bufs"])
                        else:
                            pst2 = psp.tile([128, H + 1], fp32, tag="ptr",
                                            bufs=1)
                        nc.tensor.transpose(
                            pst2[:, 0 : H + 1], pvs[:, 128 * g : 128 * (g + 1)],
                            idf[:],
                        )
                        rec = workp.tile([128, 1], fp32, tag="rec")
                        nc.vector.reciprocal(rec[:], pst2[:, H : H + 1])
                        outt = workp.tile([128, H], fp32, tag="outt")
                        nc.vector.tensor_scalar_mul(outt[:], pst2[:, 0:H], rec[:])
                        r0 = 512 * j + 128 * g
                        nc.sync.dma_start(out_d.ap()[r0 : r0 + 128, :], outt[:])

                def run_seq(*streams):
                    """Round-robin emit from several emitter lists."""
                    streams = [list(s) for s in streams]
                    while any(streams):
                        for s in streams:
                            if s:
                                s.pop(0)()

                def merged_stream(j, ctx_chunks):
                    """St pairs of slot j with this boundary's projection
                    groups injected at their dependency points: ready ctx
                    pairs first, own kv proj before the diag pairs, each new
                    ctx chunk's proj two pairs ahead of its consumers."""
                    sts = st_pair_emitters(j)
                    cp = CPAIRS[j]
                    ctx_sts = sts[:cp]
                    diag_sts = sts[cp:]
                    # ctx chunks 0..min(ctx_chunks)-1 are already projected
                    ready = min(2 * min(ctx_chunks) if ctx_chunks else cp, cp)
                    out = list(ctx_sts[: min(4, ready)])
                    out.append(lambda: kv_proj(
                        xown[j], 2 * H, ktd, 2 * j, CTXT + 4 * j, "vtd"))
                    out += diag_sts
                    out += ctx_sts[min(4, ready) : ready]
                    nxt = ready
                    for cc in ctx_chunks:
                        out.append(lambda cc=cc: ctx_proj(cc))
                        out += ctx_sts[nxt : nxt + 2]
                        nxt += 2
                    out += ctx_sts[nxt:]
                    return out

                if cfg["structure"] == "pipe":
                    # software-pipelined: slot j's PV interleaves with slot
                    # j+1's St/exp stream (with projection groups injected
                    # inline when merge_proj) so ACT never starves; the last
                    # slot's PV interleaves into its own St tail.
                    def stream(j, chunks):
                        if cfg["merge_proj"]:
                            q_proj(j)
                            return merged_stream(j, chunks)
                        own_proj(j)
                        for cc in chunks:
                            ctx_proj(cc)
                        return st_pair_emitters(j)

                    run_seq(stream(0, [0]))
                    run_seq(pv_emitters(0), stream(1, [1, 2]))
                    finalize(0)
                    run_seq(pv_emitters(1), stream(2, [3, 4]))
                    finalize(1)
                    st3 = stream(3, [5, 6])
                    n2 = len(pv_emitters(2))
                    run_seq(pv_emitters(2), st3[:n2])
                    finalize(2)
                    run_seq(pv_emitters(3), st3[n2:])
                    finalize(3)
                else:
                    def slot(j):
                        sts = st_pair_emitters(j)
                        pvs_ = pv_emitters(j)
                        if cfg["structure"] == "inter":
                            run_seq(sts, pvs_)
                        else:
                            run_seq(sts)
                            run_seq(pvs_)
                        finalize(j)

                    own_proj(0); ctx_proj(0)
                    slot(0)
                    own_proj(1); ctx_proj(1); ctx_proj(2)
                    slot(1)
                    own_proj(2); ctx_proj(3); ctx_proj(4)
                    slot(2)
                    own_proj(3); ctx_proj(5); ctx_proj(6)
                    slot(3)

            if niter == 1:
                body()
            else:
                with tc.For_i(0, niter) as iv:
                    body(iv)

    nc.compile()
    return nc


_NC_CACHE = {}


def _get_nc(niter: int = 1):
    if niter not in _NC_CACHE:
        _NC_CACHE[niter] = build_bass(niter)
    return _NC_CACHE[niter]


def make_in_maps(x, Wq, Wk, Wv):
    import ml_dtypes

    bf16 = ml_dtypes.bfloat16
    x = np.asarray(x, np.float32)
    wqs = np.asarray(Wq).T.astype(np.float32) * SCALE
    wT = np.concatenate(
        [wqs, wqs, np.asarray(Wk).T, np.asarray(Wv).T], axis=1
    ).astype(bf16)
    wT = np.ascontiguousarray(wT)

    # gate columns (pair-granular, per core type)
    need = [[4 * c for c in CHUNKS[t]] for t in range(2)]
    gates = []
    for t in range(2):
        g = np.zeros((128, NGATE), np.float32)
        for j in range(NSLOT):
            for p in range(CPAIRS[j]):
                if 2 * p >= need[t][j]:
                    g[:, GOFF[j] + p] = -50.0
        gates.append(np.ascontiguousarray(g))

    in_maps = []
    for c in range(8):
        b, t = c // 2, c % 2
        xT = np.empty((C, XCOLS), bf16)
        for j, ch in enumerate(CHUNKS[t]):
            xT[:, 512 * j : 512 * (j + 1)] = x[b, 512 * ch : 512 * (ch + 1), :].T
        xT[:, TOWN:] = x[b, 0:TCTX, :].T
        in_maps.append(
            {"xT": np.ascontiguousarray(xT), "wT": wT, "gate": gates[t]}
        )
    return in_maps


def kernel(x, Wq, Wk, Wv):
    from concourse.bass_utils import run_bass_kernel_spmd

    x = np.asarray(x, np.float32)
    nc = _get_nc(1)
    in_maps = make_in_maps(x, np.asarray(Wq), np.asarray(Wk), np.asarray(Wv))
    res = run_bass_kernel_spmd(nc, in_maps, core_ids=list(range(8)), trace=False)
    out = np.empty((B, T, H), np.float32)
    for c in range(8):
        b, t = c // 2, c % 2
        r = res.results[c]["out"]
        for j, ch in enumerate(CHUNKS[t]):
            out[b, 512 * ch : 512 * (ch + 1), :] = r[512 * j : 512 * (j + 1)]
    return out
